# revision 1
# baseline (speedup 1.0000x reference)
import sys, dataclasses
sys.path.insert(0, '/opt/trn_rl_repo')
import numpy as np
import ml_dtypes

BF = ml_dtypes.bfloat16

# dims (hardcoded per problem spec)
N, H, W, D = 2, 64, 64, 256
S = 16
NH, HD = 4, 64
MLP_H = 1024
NCORES = 8
T = 1024            # tokens per core (16 rows x 64 cols)
NT = 8              # token tiles / chunks per core
NPIX = 65 * 65      # padded image pixels
WSLOT = 64          # f32 slots per wtable row (256B)
EPS = 1e-6

_CACHE = {}


def _iotaW():
    # wtable row for (p, j) within a chunk: local ts = (j*8 + p//16)*16 + p%16
    p = np.arange(128)[:, None]; j = np.arange(16)[None, :]
    return ((j * 8 + p // 16) * 16 + p % 16).astype(np.int32)


def _iotaQ():
    p = np.arange(128)[:, None]; j = np.arange(16)[None, :]
    return (j * 8 + p // 16).astype(np.int32)


def _bcast(ap, rep):
    # insert a step-0 dim after partition dim: [P, F] -> [P, rep, F]
    return dataclasses.replace(ap, ap=[ap.ap[0], [0, rep]] + list(ap.ap[1:]))


def _build():
    import concourse.bass as bass
    import concourse.tile as tile
    from concourse.bass import IndirectOffsetOnAxis
    from concourse import bacc, mybir

    f32 = mybir.dt.float32
    bf16 = mybir.dt.bfloat16
    i16 = mybir.dt.int16
    i32 = mybir.dt.int32
    AF = mybir.ActivationFunctionType
    OP = mybir.AluOpType
    AX = mybir.AxisListType

    nc = bacc.Bacc(None, target_bir_lowering=False, debug=False)

    def din(name, shape, dt):
        return nc.dram_tensor(name, shape, dt, kind="ExternalInput")

    hsT = din("hsT", [256, T], bf16)
    hsres = din("hsres", [T, 256], f32)
    imgtable = din("imgtable", [NPIX, 256], bf16)
    phA_rhs = din("phA_rhs", [256, 44], bf16)
    phA_aug = din("phA_aug", [1, 44], bf16)
    Wtil = din("Wtil", [256, 1024], bf16)
    btil = din("btil", [1, 1024], bf16)
    WvoK = din("WvoK", [1024, 256], bf16)
    wyxvo = din("wyxvo", [1, 2048], bf16)
    acst = din("acst", [1, 256], bf16)
    W1p = din("W1p", [256, MLP_H], bf16)
    b1p = din("b1p", [1, MLP_H], bf16)
    W2w = din("W2w", [MLP_H, 256], bf16)
    b2a = din("b2a", [1, 256], bf16)
    s1rep = din("s1rep", [128, 256], f32)
    b1rep = din("b1rep", [128, 256], f32)
    s2rep = din("s2rep", [128, 256], f32)
    b2rep = din("b2rep", [128, 256], f32)
    ident = din("ident", [128, 128], f32)
    mask8f = din("mask8f", [128, 8], f32)
    bcmask = din("bcmask", [8, 128], f32)
    mask32 = din("mask32", [128, 32], bf16)
    iotaW = din("iotaW", [128, 16], i32)
    iotaQ = din("iotaQ", [128, 16], i32)
    refy = din("refy", [128, 8], f32)
    refx = din("refx", [128, 8], f32)

    out = nc.dram_tensor("out", [T, 256], f32, kind="ExternalOutput")
    wtable = nc.dram_tensor("wtable", [T * S, WSLOT], f32)
    qtable = nc.dram_tensor("qtable", [T, 1024], bf16)

    with tile.TileContext(nc) as tc:
        with tc.tile_pool(name="const", bufs=1) as cpool:
            def ld(t, shape, dt):
                x = cpool.tile(shape, dt, tag=t.name)
                nc.sync.dma_start(x[:], t.ap())
                return x

            def ldk(t, nk, cols, dt):
                xs = []
                for k in range(nk):
                    x = cpool.tile([128, cols], dt, tag=f"{t.name}_{k}")
                    nc.sync.dma_start(x[:], t.ap()[128 * k:128 * (k + 1), :])
                    xs.append(x)
                return xs

            c_hsT = ldk(hsT, 2, T, bf16)
            c_rhsA = ldk(phA_rhs, 2, 44, bf16)
            c_augA = ld(phA_aug, [1, 44], bf16)
            c_Wtil = ldk(Wtil, 2, 1024, bf16)
            c_btil = ld(btil, [1, 1024], bf16)
            c_WvoK = ldk(WvoK, 8, 256, bf16)
            c_wyxvo = ld(wyxvo, [1, 2048], bf16)
            c_acst = ld(acst, [1, 256], bf16)
            c_W1 = ldk(W1p, 2, MLP_H, bf16)
            c_b1 = ld(b1p, [1, MLP_H], bf16)
            c_W2 = ldk(W2w, 8, 256, bf16)
            c_b2 = ld(b2a, [1, 256], bf16)
            c_s1 = ld(s1rep, [128, 256], f32)
            c_b1r = ld(b1rep, [128, 256], f32)
            c_s2 = ld(s2rep, [128, 256], f32)
            c_b2r = ld(b2rep, [128, 256], f32)
            c_id = ld(ident, [128, 128], f32)
            c_m8 = ld(mask8f, [128, 8], f32)
            c_bcm = ld(bcmask, [8, 128], f32)
            c_m32 = ld(mask32, [128, 32], bf16)
            c_ioW = ld(iotaW, [128, 16], i32)
            c_ioQ = ld(iotaQ, [128, 16], i32)
            c_refy = ld(refy, [128, 8], f32)
            c_refx = ld(refx, [128, 8], f32)

            ones_bf = cpool.tile([1, 128], bf16)
            nc.vector.memset(ones_bf[:], 1.0)

            # =============== PHASE A ===============
            with (
                tc.tile_pool(name="pha", bufs=2) as apool,
                tc.tile_pool(name="phaps", bufs=1, space="PSUM") as apsum,
            ):
                for i in range(NT):
                    psA = apsum.tile([128, 44], f32, tag="psA")
                    for k in range(2):
                        nc.tensor.matmul(psA[:], c_hsT[k][:, i * 128:(i + 1) * 128],
                                         c_rhsA[k][:],
                                         start=(k == 0), stop=False)
                    nc.tensor.matmul(psA[:], ones_bf[:], c_augA[:], start=False, stop=True)

                    sgm = apool.tile([128, 32], f32, tag="sgm")
                    nc.scalar.activation(sgm[:], psA[:, 0:32], AF.Sigmoid)

                    yy = apool.tile([128, 16], f32, tag="yy")
                    xx = apool.tile([128, 16], f32, tag="xx")
                    nc.vector.tensor_scalar(yy[:], sgm[:, 0:16], 60.0, c_refy[:, i:i + 1], OP.mult, OP.add)
                    nc.vector.tensor_scalar(xx[:], sgm[:, 16:32], 60.0, c_refx[:, i:i + 1], OP.mult, OP.add)
                    nc.vector.tensor_scalar(yy[:], yy[:], 0.0, 63.0, OP.max, OP.min)
                    nc.vector.tensor_scalar(xx[:], xx[:], 0.0, 63.0, OP.max, OP.min)

                    y0i = apool.tile([128, 16], i32, tag="y0i")
                    x0i = apool.tile([128, 16], i32, tag="x0i")
                    nc.vector.tensor_copy(y0i[:], yy[:])
                    nc.vector.tensor_copy(x0i[:], xx[:])
                    y0 = apool.tile([128, 16], f32, tag="y0")
                    x0 = apool.tile([128, 16], f32, tag="x0")
                    nc.vector.tensor_copy(y0[:], y0i[:])
                    nc.vector.tensor_copy(x0[:], x0i[:])
                    fix = apool.tile([128, 16], f32, tag="fix")
                    nc.vector.tensor_tensor(fix[:], y0[:], yy[:], OP.is_gt)
                    nc.vector.tensor_tensor(y0[:], y0[:], fix[:], OP.subtract)
                    nc.vector.tensor_tensor(fix[:], x0[:], xx[:], OP.is_gt)
                    nc.vector.tensor_tensor(x0[:], x0[:], fix[:], OP.subtract)

                    wy = apool.tile([128, 16], f32, tag="wy")
                    wx = apool.tile([128, 16], f32, tag="wx")
                    nc.vector.tensor_tensor(wy[:], yy[:], y0[:], OP.subtract)
                    nc.vector.tensor_tensor(wx[:], xx[:], x0[:], OP.subtract)
                    omy = apool.tile([128, 16], f32, tag="omy")
                    omx = apool.tile([128, 16], f32, tag="omx")
                    nc.vector.tensor_scalar(omy[:], wy[:], -1.0, 1.0, OP.mult, OP.add)
                    nc.vector.tensor_scalar(omx[:], wx[:], -1.0, 1.0, OP.mult, OP.add)

                    br = apool.tile([128, S * WSLOT], f32, tag="bridge")
                    brv = br[:].rearrange("p (s w) -> p s w", w=WSLOT)
                    nc.gpsimd.memset(brv[:, :, 20:WSLOT], 0.0)
                    nc.vector.tensor_tensor(brv[:, :, 0], omy[:], omx[:], OP.mult)
                    nc.vector.tensor_tensor(brv[:, :, 1], omy[:], wx[:], OP.mult)
                    nc.vector.tensor_tensor(brv[:, :, 2], wy[:], omx[:], OP.mult)
                    nc.vector.tensor_tensor(brv[:, :, 3], wy[:], wx[:], OP.mult)
                    nc.vector.tensor_scalar(brv[:, :, 4], sgm[:, 0:16], 60.0, -30.0, OP.mult, OP.add)
                    nc.vector.tensor_scalar(brv[:, :, 5], sgm[:, 16:32], 60.0, -30.0, OP.mult, OP.add)
                    # gy(4) gx(4) ctil(4): replicate psA[:,32:44] along s via step-0 src AP
                    nc.vector.tensor_copy(brv[:, :, 6:18], _bcast(psA[:, 32:44], S))

                    nc.vector.scalar_tensor_tensor(brv[:, :, 18], y0[:], 65.0, x0[:], OP.mult, OP.add)
                    nc.vector.tensor_scalar(brv[:, :, 19], brv[:, :, 18], 65.0, None, OP.add)
                    nc.sync.dma_start(
                        wtable.ap()[i * 2048:(i + 1) * 2048, :].rearrange("(p s) w -> p (s w)", p=128),
                        br[:])

                    psQ = apsum.tile([128, 1024], f32, tag="psQ")
                    for nn in range(2):
                        for k in range(2):
                            nc.tensor.matmul(psQ[:, nn * 512:(nn + 1) * 512],
                                             c_hsT[k][:, i * 128:(i + 1) * 128],
                                             c_Wtil[k][:, nn * 512:(nn + 1) * 512],
                                             start=(k == 0), stop=False)
                        nc.tensor.matmul(psQ[:, nn * 512:(nn + 1) * 512], ones_bf[:],
                                         c_btil[:, nn * 512:(nn + 1) * 512], start=False, stop=True)
                    qsb = apool.tile([128, 1024], bf16, tag="qsb")
                    nc.scalar.copy(qsb[:], psQ[:])
                    nc.sync.dma_start(qtable.ap()[i * 128:(i + 1) * 128, :], qsb[:])

            import os as _os
            _STOP = int(_os.environ.get("KSTOP", "0"))
            # =============== PHASE B ===============
            with (
                tc.tile_pool(name="gat", bufs=2) as gpool,
                tc.tile_pool(name="gatq", bufs=1) as qpool,
                tc.tile_pool(name="chk", bufs=2) as kpool,
                tc.tile_pool(name="chps", bufs=1, space="PSUM") as kpsum,
            ):
                for g in range(NT):
                    if _STOP == 1:
                        yz = kpool.tile([128, 256], f32, tag="yz")
                        nc.sync.dma_start(yz[:], hsres.ap()[g * 128:(g + 1) * 128, :])
                        nc.sync.dma_start(out.ap()[g * 128:(g + 1) * 128, :], yz[:])
                        continue
                    WB = kpool.tile([128, S, WSLOT], f32, tag="WB")
                    for j in range(S):
                        nc.gpsimd.indirect_dma_start(
                            WB[:, j, :], None, wtable.ap(),
                            IndirectOffsetOnAxis(ap=c_ioW[:, j:j + 1], axis=0),
                            element_offset=g * 2048 * WSLOT)
                    ofsA = kpool.tile([128, S], i32, tag="ofsA")
                    ofsB = kpool.tile([128, S], i32, tag="ofsB")
                    wbv2 = WB[:].rearrange("p s w -> p (s w)")
                    ofsf = kpool.tile([128, S], f32, tag="ofsf")
                    nc.vector.tensor_scalar(
                        ofsf[:], dataclasses.replace(wbv2, offset=wbv2.offset + 18,
                                                     ap=[wbv2.ap[0], [WSLOT, S]]),
                        0.0, 4158.0, OP.max, OP.min)
                    nc.vector.tensor_copy(ofsA[:], ofsf[:])
                    nc.vector.tensor_scalar(ofsf[:], ofsf[:], 65.0, None, OP.add)
                    nc.vector.tensor_copy(ofsB[:], ofsf[:])
                    ga = gpool.tile([128, S, 512], bf16, tag="ga")
                    gb = gpool.tile([128, S, 512], bf16, tag="gb")
                    QR = qpool.tile([128, S, 1024], bf16, tag="QR")
                    for j in range(S):
                        nc.gpsimd.indirect_dma_start(
                            ga[:, j, :], None, imgtable.ap(),
                            IndirectOffsetOnAxis(ap=ofsA[:, j:j + 1], axis=0))
                        nc.gpsimd.indirect_dma_start(
                            gb[:, j, :], None, imgtable.ap(),
                            IndirectOffsetOnAxis(ap=ofsB[:, j:j + 1], axis=0))
                        nc.gpsimd.indirect_dma_start(
                            QR[:, j, :], None, qtable.ap(),
                            IndirectOffsetOnAxis(ap=c_ioQ[:, j:j + 1], axis=0),
                            element_offset=g * 128 * 1024)
                    cors = [ga[:].rearrange("p s (c d) -> p s c d", c=2),
                            gb[:].rearrange("p s (c d) -> p s c d", c=2)]

                    if _STOP == 2:
                        yz = kpool.tile([128, 256], f32, tag="yz")
                        nc.sync.dma_start(yz[:], hsres.ap()[g * 128:(g + 1) * 128, :])
                        nc.vector.tensor_tensor(yz[:], yz[:], ga[:, 0, 0:256], OP.add)
                        nc.vector.tensor_tensor(yz[:], yz[:], gb[:, 0, 0:256], OP.add)
                        nc.vector.tensor_tensor(yz[:], yz[:], QR[:, 0, 0:256], OP.add)
                        nc.vector.scalar_tensor_tensor(yz[:], yz[:], WB[:, 0, 0:1], yz[:], OP.mult, OP.add)
                        nc.sync.dma_start(out.ap()[g * 128:(g + 1) * 128, :], yz[:])
                        continue
                    kv = kpool.tile([128, S, 256], bf16, tag="kv")
                    sc = kpool.tile([128, 64], f32, tag="sc")
                    comb = kpool.tile([128, S, 4], f32, tag="comb")
                    scr = kpool.tile([128, 256], bf16, tag="scr")
                    scv = sc[:].rearrange("p (s a) -> p s a", a=4)

                    for j in range(S):
                        nc.vector.tensor_scalar(kv[:, j, :], cors[0][:, j, 0, :], WB[:, j, 0:1], None, OP.mult)
                        nc.vector.scalar_tensor_tensor(kv[:, j, :], cors[0][:, j, 1, :], WB[:, j, 1:2], kv[:, j, :], OP.mult, OP.add)
                        nc.vector.scalar_tensor_tensor(kv[:, j, :], cors[1][:, j, 0, :], WB[:, j, 2:3], kv[:, j, :], OP.mult, OP.add)
                        nc.vector.scalar_tensor_tensor(kv[:, j, :], cors[1][:, j, 1, :], WB[:, j, 3:4], kv[:, j, :], OP.mult, OP.add)
                        nc.vector.scalar_tensor_tensor(comb[:, j, :], WB[:, j, 6:10], WB[:, j, 4:5], WB[:, j, 14:18], OP.mult, OP.add)
                        nc.vector.scalar_tensor_tensor(comb[:, j, :], WB[:, j, 10:14], WB[:, j, 5:6], comb[:, j, :], OP.mult, OP.add)
                        if _STOP == 3:
                            continue
                        for a in range(4):
                            nc.vector.scalar_tensor_tensor(
                                scr[:], QR[:, j, 256 * a:256 * (a + 1)], 1.0, kv[:, j, :],
                                OP.mult, OP.mult,
                                accum_out=scv[:, j, a:a + 1])

                    nc.vector.tensor_tensor(sc[:], sc[:], comb[:].rearrange("p s a -> p (s a)"), OP.add)
                    if _STOP in (3, 4):
                        yz = kpool.tile([128, 256], f32, tag="yz")
                        nc.sync.dma_start(yz[:], hsres.ap()[g * 128:(g + 1) * 128, :])
                        nc.vector.tensor_tensor(yz[:], yz[:], kv[:, 0, :], OP.add)
                        nc.vector.scalar_tensor_tensor(yz[:], yz[:], comb[:, 0, 0:1], yz[:], OP.mult, OP.add)
                        if _STOP == 4:
                            nc.vector.scalar_tensor_tensor(yz[:], yz[:], sc[:, 0:1], yz[:], OP.mult, OP.add)
                        nc.sync.dma_start(out.ap()[g * 128:(g + 1) * 128, :], yz[:])
                        continue
                    pe_u = kpool.tile([128, 64], f32, tag="pe_u")
                    nc.scalar.activation(pe_u[:], sc[:], AF.Exp)
                    psZ = kpsum.tile([8, 64], f32, tag="psmA")
                    nc.tensor.matmul(psZ[:], c_m8[:], pe_u[:], start=True, stop=True)
                    rz = kpool.tile([8, 64], f32, tag="rz")
                    nc.vector.reciprocal(rz[:], psZ[:])
                    psR = kpsum.tile([128, 64], f32, tag="psmA")
                    nc.tensor.matmul(psR[:], c_bcm[:], rz[:], start=True, stop=True)
                    pn = kpool.tile([128, 64], bf16, tag="pn")
                    nc.vector.tensor_tensor(pn[:], pe_u[:], psR[:], OP.mult)

                    oyxall = kpool.tile([128, S * 2], bf16, tag="oyxall")
                    wbv = WB[:].rearrange("p s w -> p (s w)")
                    nc.vector.tensor_copy(
                        oyxall[:],
                        dataclasses.replace(wbv, offset=wbv.offset + 4,
                                            ap=[wbv.ap[0], [WSLOT, S], [1, 2]]))

                    ct0 = kpsum.tile([128, S * 32], f32, tag="ct0")
                    ct1 = kpsum.tile([128, S * 32], f32, tag="ct1")
                    pyxY = kpsum.tile([1, S * 32], f32, tag="pyxY")
                    pyxX = kpsum.tile([1, S * 32], f32, tag="pyxX")
                    for j in range(S):
                        pmj = kpool.tile([128, 32], bf16, tag="pm")
                        nc.vector.tensor_tensor(
                            pmj[:], _bcast(pn[:, j * 4:(j + 1) * 4], 8), c_m32[:], OP.mult)
                        nc.tensor.matmul(ct0[:, j * 32:(j + 1) * 32], kv[:, j, 0:128], pmj[:],
                                         start=True, stop=True)
                        nc.tensor.matmul(ct1[:, j * 32:(j + 1) * 32], kv[:, j, 128:256], pmj[:],
                                         start=True, stop=True)
                        nc.tensor.matmul(pyxY[:, j * 32:(j + 1) * 32],
                                         oyxall[:, j * 2:j * 2 + 1], pmj[:],
                                         start=True, stop=True)
                        nc.tensor.matmul(pyxX[:, j * 32:(j + 1) * 32],
                                         oyxall[:, j * 2 + 1:j * 2 + 2], pmj[:],
                                         start=True, stop=True)

                    ct0s = kpool.tile([128, S * 32], bf16, tag="ct0s")
                    ct1s = kpool.tile([128, S * 32], bf16, tag="ct1s")
                    pyxYs = kpool.tile([1, S * 32], bf16, tag="pyxYs")
                    pyxXs = kpool.tile([1, S * 32], bf16, tag="pyxXs")
                    nc.scalar.copy(ct0s[:], ct0[:])
                    nc.vector.tensor_copy(ct1s[:], ct1[:])
                    nc.vector.tensor_copy(pyxYs[:], pyxY[:])
                    nc.vector.tensor_copy(pyxXs[:], pyxX[:])

                    psAt = kpsum.tile([128, 256], f32, tag="psmB")
                    first = True
                    for a in range(4):
                        for h in range(2):
                            cts = (ct0s, ct1s)[h]
                            lh = cts[:].rearrange("p (j t a) -> p (j t) a", j=S, a=4)
                            nc.tensor.matmul(psAt[:], lh[:, :, a],
                                             c_WvoK[a * 2 + h][:],
                                             start=first, stop=False)
                            first = False
                    for yx, pys in enumerate((pyxYs, pyxXs)):
                        lhp = pys[:].rearrange("p (j t a) -> p (j t) a", j=S, a=4)
                        for a in range(4):
                            nc.tensor.matmul(psAt[:], lhp[:, :, a],
                                             c_wyxvo[:, (yx * 4 + a) * 256:(yx * 4 + a + 1) * 256],
                                             start=False, stop=False)
                    nc.tensor.matmul(psAt[:], ones_bf[:], c_acst[:], start=False, stop=True)

                    hsr = kpool.tile([128, 256], f32, tag="hsr")
                    nc.sync.dma_start(hsr[:], hsres.ap()[g * 128:(g + 1) * 128, :])
                    xr = kpool.tile([128, 256], f32, tag="xr")
                    nc.vector.tensor_tensor(xr[:], hsr[:], psAt[:], OP.add)

                    def layernorm(xin, tagp):
                        sq = kpool.tile([128, 256], f32, tag="lnsq")
                        ssq = kpool.tile([128, 1], f32, tag=tagp + "ssq")
                        nc.scalar.activation(sq[:], xin[:], AF.Square, accum_out=ssq[:])
                        sm = kpool.tile([128, 1], f32, tag=tagp + "sm")
                        nc.vector.tensor_reduce(sm[:], xin[:], axis=AX.X, op=OP.add)
                        mn = kpool.tile([128, 1], f32, tag=tagp + "mn")
                        nc.vector.tensor_scalar(mn[:], sm[:], 1.0 / 256.0, None, OP.mult)
                        msq = kpool.tile([128, 1], f32, tag=tagp + "msq")
                        nc.vector.tensor_tensor(msq[:], mn[:], mn[:], OP.mult)
                        vr = kpool.tile([128, 1], f32, tag=tagp + "vr")
                        nc.vector.scalar_tensor_tensor(vr[:], ssq[:], 1.0 / 256.0, msq[:], OP.mult, OP.subtract)
                        nc.vector.tensor_scalar(vr[:], vr[:], EPS, None, OP.add)
                        sd = kpool.tile([128, 1], f32, tag=tagp + "sd")
                        nc.scalar.activation(sd[:], vr[:], AF.Sqrt)
                        rstd = kpool.tile([128, 1], f32, tag=tagp + "rstd")
                        nc.vector.reciprocal(rstd[:], sd[:])
                        xo = kpool.tile([128, 256], f32, tag=tagp + "xo")
                        nc.vector.tensor_scalar(xo[:], xin[:], mn[:], rstd[:], OP.subtract, OP.mult)
                        return xo

                    xh = layernorm(xr, "ln1")
                    x1 = kpool.tile([128, 256], f32, tag="x1")
                    nc.vector.tensor_tensor(x1[:], xh[:], c_s1[:], OP.mult)
                    nc.vector.tensor_tensor(x1[:], x1[:], c_b1r[:], OP.add)

                    # transpose xh -> xT [256ch(2x128), 128t] bf16
                    psX = kpsum.tile([128, 256], f32, tag="psmA")
                    nc.tensor.transpose(psX[:, 0:128], xh[:, 0:128], c_id[:])
                    nc.tensor.transpose(psX[:, 128:256], xh[:, 128:256], c_id[:])
                    xT = kpool.tile([128, 256], bf16, tag="xT")
                    nc.scalar.copy(xT[:], psX[:])

                    # hT = W1'.T @ x1: psH[mi] [128, 4x128] for m = mi*4+q
                    psH0 = kpsum.tile([128, 512], f32, tag="psH0")
                    psH1 = kpsum.tile([128, 512], f32, tag="psH1")
                    psH = [psH0, psH1]
                    for m in range(8):
                        pd = psH[m // 4][:, (m % 4) * 128:(m % 4 + 1) * 128]
                        for k in range(2):
                            nc.tensor.matmul(pd, c_W1[k][:, m * 128:(m + 1) * 128],
                                             xT[:, 128 * k:128 * (k + 1)],
                                             start=(k == 0), stop=False)
                        nc.tensor.matmul(pd, c_b1[:, m * 128:(m + 1) * 128], ones_bf[:],
                                         start=False, stop=True)
                    gh = kpool.tile([128, MLP_H], bf16, tag="gh")
                    gsq = kpool.tile([128, 512], f32, tag="gsq")
                    gu = kpool.tile([128, 512], f32, tag="gu")
                    for mi in range(2):
                        hv = psH[mi][:]
                        nc.scalar.activation(gsq[:], hv, AF.Square)
                        nc.vector.tensor_scalar(gsq[:], gsq[:], 0.044715, 1.0, OP.mult, OP.add)
                        nc.vector.tensor_tensor(gu[:], gsq[:], hv, OP.mult)
                        nc.scalar.activation(gu[:], gu[:], AF.Tanh, scale=0.7978845608028654)
                        nc.vector.tensor_scalar(gu[:], gu[:], 0.5, 0.5, OP.mult, OP.add)
                        nc.vector.tensor_tensor(gh[:, mi * 512:(mi + 1) * 512], gu[:], hv, OP.mult)

                    psY = kpsum.tile([128, 256], f32, tag="psmB")
                    for k in range(8):
                        nc.tensor.matmul(psY[:], gh[:, k * 128:(k + 1) * 128],
                                         c_W2[k][:],
                                         start=(k == 0), stop=False)
                    nc.tensor.matmul(psY[:], ones_bf[:], c_b2[:], start=False, stop=True)

                    z = kpool.tile([128, 256], f32, tag="z")
                    nc.vector.tensor_tensor(z[:], x1[:], psY[:], OP.add)
                    xh2 = layernorm(z, "ln2")
                    yout = kpool.tile([128, 256], f32, tag="yout")
                    nc.vector.tensor_tensor(yout[:], xh2[:], c_s2[:], OP.mult)
                    nc.vector.tensor_tensor(yout[:], yout[:], c_b2r[:], OP.add)
                    nc.sync.dma_start(out.ap()[g * 128:(g + 1) * 128, :], yout[:])

    nc.compile()
    return nc


def _host_prep(inputs):
    f = np.float32
    hs = np.asarray(inputs['hidden_state'], f)
    ehs = np.asarray(inputs['embedded_hidden_state'], f)
    W_off = np.asarray(inputs['W_off'], f)      # [D, S, 2]
    b_off = np.asarray(inputs['b_off'], f)      # [S, 2]
    W_kvp = np.asarray(inputs['W_kvp'], f)      # [2, D]
    b_kvp = np.asarray(inputs['b_kvp'], f)      # [D]
    Wq = np.asarray(inputs['Wq'], f); bq = np.asarray(inputs['bq'], f)
    Wk = np.asarray(inputs['Wk'], f); bk = np.asarray(inputs['bk'], f)
    Wv = np.asarray(inputs['Wv'], f); bv = np.asarray(inputs['bv'], f)
    Wo = np.asarray(inputs['Wo'], f); bo = np.asarray(inputs['bo'], f)
    ln1_s = np.asarray(inputs['ln1_s'], f); ln1_b = np.asarray(inputs['ln1_b'], f)
    W1 = np.asarray(inputs['W1'], f); b1 = np.asarray(inputs['b1'], f)
    W2 = np.asarray(inputs['W2'], f); b2 = np.asarray(inputs['b2'], f)
    ln2_s = np.asarray(inputs['ln2_s'], f); ln2_b = np.asarray(inputs['ln2_b'], f)

    sc = 1.0 / np.sqrt(HD)
    # folded weights (shared)
    Wtil = np.zeros((256, 4, 256), f)
    btilv = np.zeros((4, 256), f)
    gyv = np.zeros((256, 4), f); gxv = np.zeros((256, 4), f); cv = np.zeros((256, 4), f)
    gyb = np.zeros(4, f); gxb = np.zeros(4, f); cb = np.zeros(4, f)
    WvoK = np.zeros((4, 256, 256), f)
    wyxvo = np.zeros((8, 256), f)
    acst = np.array(bo, f)
    for a in range(4):
        Wt = (Wq[:, a, :] @ Wk[:, a, :].T) * sc      # [256,256]
        bt = (bq[a] @ Wk[:, a, :].T) * sc            # [256]
        Wtil[:, a, :] = Wt
        btilv[a] = bt
        gyv[:, a] = Wt @ W_kvp[0]; gyb[a] = bt @ W_kvp[0]
        gxv[:, a] = Wt @ W_kvp[1]; gxb[a] = bt @ W_kvp[1]
        kb = b_kvp @ Wk[:, a, :] + bk[a]             # [64]
        cv[:, a] = (Wq[:, a, :] @ kb) * sc
        cb[a] = (bq[a] @ kb) * sc
        Wvo = Wv[:, a, :] @ Wo[a]                    # [256, 256]
        WvoK[a] = Wvo
        wyxvo[a] = W_kvp[0] @ Wvo
        wyxvo[4 + a] = W_kvp[1] @ Wvo
        acst = acst + (b_kvp @ Wv[:, a, :] + bv[a]) @ Wo[a]

    Woff_flat = np.concatenate(
        [W_off[:, :, 0], W_off[:, :, 1], gyv, gxv, cv], axis=1)    # [256, 44]
    baug = np.concatenate(
        [b_off[:, 0], b_off[:, 1], gyb, gxb, cb])[None, :]         # [1, 44]
    W1p = ln1_s[:, None] * W1
    b1p = (ln1_b @ W1 + b1)[None, :]

    shared = {
        'phA_rhs': Woff_flat.astype(BF), 'phA_aug': baug.astype(BF),
        'Wtil': Wtil.reshape(256, 1024).astype(BF),
        'btil': btilv.reshape(1, 1024).astype(BF),
        'WvoK': WvoK.reshape(1024, 256).astype(BF),
        'wyxvo': wyxvo.reshape(1, 2048).astype(BF), 'acst': acst[None, :].astype(BF),
        'W1p': W1p.astype(BF), 'b1p': b1p.astype(BF),
        'W2w': W2.astype(BF), 'b2a': b2[None, :].astype(BF),
        's1rep': np.tile(ln1_s, (128, 1)).astype(f),
        'b1rep': np.tile(ln1_b, (128, 1)).astype(f),
        's2rep': np.tile(ln2_s, (128, 1)).astype(f),
        'b2rep': np.tile(ln2_b, (128, 1)).astype(f),
        'ident': np.eye(128, dtype=f),
        'mask8f': np.repeat(np.eye(8, dtype=f), 16, axis=0),
        'bcmask': np.repeat(np.eye(8, dtype=f), 16, axis=0).T.copy(),
        'mask32': np.repeat(np.repeat(np.eye(8, dtype=f), 16, axis=0), 4, axis=1).astype(BF),
        'iotaW': _iotaW(), 'iotaQ': _iotaQ(),
    }
    # per-tile reference grid: token p of tile i -> global token i*128+p
    tok = np.arange(T)
    refy_all = (tok // 64).astype(f)  # local h in [0,16)
    refx_all = (tok % 64).astype(f) - 30.0

    in_maps = []
    for c in range(NCORES):
        n, r0 = c // 4, (c % 4) * 16
        hs_c = hs[n, r0:r0 + 16].reshape(T, 256)
        img = ehs[n]
        P = np.zeros((65, 65, 256), f)
        P[:64, :64] = img
        P[64, :64] = img[63]
        P[:64, 64] = P[:64, 63]
        P[64, 64] = img[63, 63]
        m = dict(shared)
        m['hsT'] = np.ascontiguousarray(hs_c.T).astype(BF)
        m['hsres'] = hs_c.copy()
        m['imgtable'] = P.reshape(NPIX, 256).astype(BF)
        m['refy'] = np.ascontiguousarray(
            (refy_all + r0 - 30.0).reshape(8, 128).T)
        m['refx'] = np.ascontiguousarray(refx_all.reshape(8, 128).T)
        in_maps.append(m)
    return in_maps


def kernel(**inputs):
    from concourse.bass_utils import run_bass_kernel_spmd
    if 'nc' not in _CACHE:
        _CACHE['nc'] = _build()
    nc = _CACHE['nc']
    in_maps = _host_prep(inputs)
    res = run_bass_kernel_spmd(nc, in_maps, list(range(NCORES)))
    outs = [res.results[c]['out'].reshape(16, 64, 256) for c in range(NCORES)]
    full = np.zeros((N, H, W, D), np.float32)
    for c in range(NCORES):
        full[c // 4, (c % 4) * 16:(c % 4) * 16 + 16] = outs[c]
    return full



# revision 27
# speedup vs baseline: 1.2567x; 1.2567x over previous
import sys, dataclasses
sys.path.insert(0, '/opt/trn_rl_repo')
import numpy as np
import ml_dtypes

BF = ml_dtypes.bfloat16

# dims (hardcoded per problem spec)
N, H, W, D = 2, 64, 64, 256
S = 16
NH, HD = 4, 64
MLP_H = 1024
NCORES = 8
T = 1024            # tokens per core (16 rows x 64 cols)
NT = 8              # chunks of 128 tokens
NPIX = 65 * 65      # padded image pixels
WSLOT = 20          # f32 slots per wtable row (18 = pix idx)
EPS = 1e-6

# phase-B layout (Q16): partition p = (t8=p//16, q=p%16); token = 8*j + p//16
# (j in [0,16)); sample s = p%16.
_CACHE = {}


def _bcast(ap, rep):
    # insert a step-0 dim after partition dim: [P, F] -> [P, rep, F]
    return dataclasses.replace(ap, ap=[ap.ap[0], [0, rep]] + list(ap.ap[1:]))


def _rp(ap, off, axes):
    # keep partition axis, replace free axes; offset in elements
    return dataclasses.replace(ap, offset=ap.offset + off,
                               ap=[ap.ap[0]] + [list(a) for a in axes])


def _build():
    import concourse.bass as bass
    import concourse.tile as tile
    from concourse import bacc, mybir, library_config

    f32 = mybir.dt.float32
    bf16 = mybir.dt.bfloat16
    i32 = mybir.dt.int32
    i16 = mybir.dt.int16
    AF = mybir.ActivationFunctionType
    OP = mybir.AluOpType
    AX = mybir.AxisListType

    nc = bacc.Bacc(None, target_bir_lowering=False, debug=False)

    def din(name, shape, dt):
        return nc.dram_tensor(name, shape, dt, kind="ExternalInput")

    hsT = din("hsT", [256, T], bf16)
    hsT2 = din("hsT2", [256, T], bf16)
    hsres = din("hsres", [T, 256], f32)
    imgquad = din("imgquad", [4160, 1024], bf16)
    phA_rhs = din("phA_rhs", [256, 44], bf16)
    phA_aug = din("phA_aug", [1, 44], bf16)
    Wtil = din("Wtil", [256, 1024], bf16)
    btil = din("btil", [1, 1024], bf16)
    WvoK = din("WvoK", [1024, 256], bf16)
    wyxvo = din("wyxvo", [1, 2048], bf16)
    acst = din("acst", [1, 256], bf16)
    W1p = din("W1p", [256, MLP_H], bf16)
    b1p = din("b1p", [1, MLP_H], bf16)
    W2w = din("W2w", [MLP_H, 256], bf16)
    b2a = din("b2a", [1, 256], bf16)
    s1rep = din("s1rep", [128, 256], f32)
    b1rep = din("b1rep", [128, 256], f32)
    s2rep = din("s2rep", [128, 256], f32)
    b2rep = din("b2rep", [128, 256], f32)
    ident = din("ident", [128, 128], f32)
    mask8f = din("mask8f", [128, 8], f32)
    bcmask = din("bcmask", [8, 128], f32)
    mask32 = din("mask32", [128, 32], bf16)
    ioQ2 = din("ioQ2", [128, 1], i32)
    refy = din("refy", [128, 8], f32)
    refx = din("refx", [128, 8], f32)

    out = nc.dram_tensor("out", [T, 256], f32, kind="ExternalOutput")
    wtab = [nc.dram_tensor(f"wtab{g}", [128 * S, WSLOT], f32) for g in range(NT)]
    qtable = nc.dram_tensor("qtable", [T, 1024], bf16)

    with tile.TileContext(nc) as tc:
        with tc.tile_pool(name="const", bufs=1) as cpool:
            def ld(t, shape, dt):
                x = cpool.tile(shape, dt, tag=t.name)
                nc.sync.dma_start(x[:], t.ap())
                return x

            def ldk(t, nk, cols, dt):
                xs = []
                for k in range(nk):
                    x = cpool.tile([128, cols], dt, tag=f"{t.name}_{k}")
                    nc.sync.dma_start(x[:], t.ap()[128 * k:128 * (k + 1), :])
                    xs.append(x)
                return xs

            c_hsT = ldk(hsT, 2, T, bf16)
            c_hsT2 = ldk(hsT2, 2, T, bf16)
            c_rhsA = ldk(phA_rhs, 2, 44, bf16)
            c_augA = ld(phA_aug, [1, 44], bf16)
            c_Wtil = ldk(Wtil, 2, 1024, bf16)
            c_btil = ld(btil, [1, 1024], bf16)
            c_WvoK = ldk(WvoK, 8, 256, bf16)
            c_wyxvo = ld(wyxvo, [1, 2048], bf16)
            c_acst = ld(acst, [1, 256], bf16)
            c_W1 = ldk(W1p, 2, MLP_H, bf16)
            c_b1 = ld(b1p, [1, MLP_H], bf16)
            c_W2 = ldk(W2w, 8, 256, bf16)
            c_b2 = ld(b2a, [1, 256], bf16)
            c_s1 = ld(s1rep, [128, 256], f32)
            c_b1r = ld(b1rep, [128, 256], f32)
            c_s2 = ld(s2rep, [128, 256], f32)
            c_b2r = ld(b2rep, [128, 256], f32)
            c_id = ld(ident, [128, 128], f32)
            c_m8 = ld(mask8f, [128, 8], f32)
            c_bcm = ld(bcmask, [8, 128], f32)
            c_m32 = ld(mask32, [128, 32], bf16)
            c_ioQ = ld(ioQ2, [128, 1], i32)
            c_refy = ld(refy, [128, 8], f32)
            c_refx = ld(refx, [128, 8], f32)

            ones_bf = cpool.tile([1, 128], bf16)
            nc.vector.memset(ones_bf[:], 1.0)

            # =============== PHASE A ===============
            with (
                tc.tile_pool(name="pha", bufs=2) as apool,
                tc.tile_pool(name="phaps", bufs=1, space="PSUM") as apsum,
            ):
                for i in range(NT):
                    psA = apsum.tile([128, 44], f32, tag="psA")
                    for k in range(2):
                        nc.tensor.matmul(psA[:], c_hsT[k][:, i * 128:(i + 1) * 128],
                                         c_rhsA[k][:],
                                         start=(k == 0), stop=False)
                    nc.tensor.matmul(psA[:], ones_bf[:], c_augA[:], start=False, stop=True)

                    sgm = apool.tile([128, 32], f32, tag="sgm")
                    nc.scalar.activation(sgm[:], psA[:, 0:32], AF.Sigmoid)

                    # merged y|x coord pipeline [128, 32] (cols 0:16 y, 16:32 x)
                    yx = apool.tile([128, 32], f32, tag="yx")
                    nc.vector.tensor_scalar(yx[:, 0:16], sgm[:, 0:16], 60.0, c_refy[:, i:i + 1], OP.mult, OP.add)
                    nc.vector.tensor_scalar(yx[:, 16:32], sgm[:, 16:32], 60.0, c_refx[:, i:i + 1], OP.mult, OP.add)
                    nc.vector.tensor_scalar(yx[:], yx[:], 0.0, 63.0, OP.max, OP.min)
                    fi = apool.tile([128, 32], i32, tag="fi")
                    f0 = apool.tile([128, 32], f32, tag="f0")
                    nc.vector.tensor_copy(fi[:], yx[:])
                    nc.vector.tensor_copy(f0[:], fi[:])
                    fix = apool.tile([128, 32], f32, tag="fix")
                    nc.vector.tensor_tensor(fix[:], f0[:], yx[:], OP.is_gt)
                    nc.vector.tensor_tensor(f0[:], f0[:], fix[:], OP.subtract)
                    wgt = apool.tile([128, 32], f32, tag="wgt")
                    omw = apool.tile([128, 32], f32, tag="omw")
                    nc.vector.tensor_tensor(wgt[:], yx[:], f0[:], OP.subtract)
                    nc.vector.tensor_scalar(omw[:], wgt[:], -1.0, 1.0, OP.mult, OP.add)

                    br = apool.tile([128, S, WSLOT], f32, tag="bridge")
                    wy, wx = wgt[:, 0:16], wgt[:, 16:32]
                    oy, ox = omw[:, 0:16], omw[:, 16:32]
                    nc.vector.tensor_tensor(br[:, :, 0], oy, ox, OP.mult)
                    nc.vector.tensor_tensor(br[:, :, 1], oy, wx, OP.mult)
                    nc.vector.tensor_tensor(br[:, :, 2], wy, ox, OP.mult)
                    nc.vector.tensor_tensor(br[:, :, 3], wy, wx, OP.mult)
                    nc.vector.tensor_scalar(br[:, :, 4], sgm[:, 0:16], 60.0, -30.0, OP.mult, OP.add)
                    nc.vector.tensor_scalar(br[:, :, 5], sgm[:, 16:32], 60.0, -30.0, OP.mult, OP.add)
                    nc.vector.tensor_copy(br[:, :, 6:18], _bcast(psA[:, 32:44], S))
                    nc.vector.scalar_tensor_tensor(br[:, :, 18], f0[:, 0:16], 65.0, f0[:, 16:32], OP.mult, OP.add)
                    nc.vector.memset(br[:, :, 19], 0.0)
                    nc.sync.dma_start(
                        wtab[i].ap().rearrange("(p s) w -> p (s w)", p=128),
                        br[:])

                    psQ = apsum.tile([128, 1024], f32, tag="psQ")
                    for nn in range(2):
                        for k in range(2):
                            nc.tensor.matmul(psQ[:, nn * 512:(nn + 1) * 512],
                                             c_hsT2[k][:, i * 128:(i + 1) * 128],
                                             c_Wtil[k][:, nn * 512:(nn + 1) * 512],
                                             start=(k == 0), stop=False)
                        nc.tensor.matmul(psQ[:, nn * 512:(nn + 1) * 512], ones_bf[:],
                                         c_btil[:, nn * 512:(nn + 1) * 512], start=False, stop=True)
                    qsb = apool.tile([128, 1024], bf16, tag="qsb")
                    nc.scalar.copy(qsb[:], psQ[:])
                    nc.sync.dma_start(qtable.ap()[i * 128:(i + 1) * 128, :], qsb[:])

            # =============== PHASE B ===============
            with (
                tc.tile_pool(name="gat", bufs=2) as gpool,
                tc.tile_pool(name="qg", bufs=1) as qpool,
                tc.tile_pool(name="chk", bufs=2) as kpool,
                tc.tile_pool(name="scr", bufs=1) as spool,
                tc.tile_pool(name="chps", bufs=1, space="PSUM") as kpsum,
            ):
                from concourse.bass import IndirectOffsetOnAxis
                import os as _os
                KQS = int(_os.environ.get("KQSPLIT", "1"))
                KSTOP = int(_os.environ.get("KSTOP", "0"))
                for g in range(NT):
                    # --- query gather: per-partition contiguous rows ---
                    QRb = qpool.tile([128, S, 1024], bf16, tag="QRb")
                    nj = S // KQS
                    for sub in range(KQS):
                        qd = QRb[:, sub * nj:(sub + 1) * nj, :]
                        qd = dataclasses.replace(
                            qd, ap=[qd.ap[0], [1, nj * 1024]])
                        nc.gpsimd.indirect_dma_start(
                            qd, None, qtable.ap(),
                            IndirectOffsetOnAxis(ap=c_ioQ[:], axis=0),
                            element_offset=(g * 128 + sub * nj) * 1024)
                    if KSTOP == 20:
                        yz = kpool.tile([128, 256], f32, tag="yz")
                        nc.scalar.dma_start(yz[:], hsres.ap()[g * 128:(g + 1) * 128, :])
                        nc.vector.tensor_tensor(yz[:], yz[:], QRb[:, 0, 0:256], OP.add)
                        nc.sync.dma_start(out.ap()[g * 128:(g + 1) * 128, :], yz[:])
                        continue

                    # --- wtable load: row = 128*j + p, affine ---
                    WB = kpool.tile([128, S, WSLOT], f32, tag="WB")
                    wsrc = dataclasses.replace(
                        wtab[g].ap(),
                        ap=[[WSLOT, 128], [128 * WSLOT, 16], [1, WSLOT]])
                    nc.sync.dma_start(WB[:], wsrc)
                    WBv = WB[:].rearrange("p u w -> p (u w)")

                    # --- corner offsets from wtable slot 18 ---
                    ofsf = kpool.tile([128, 16], f32, tag="ofsf")
                    ofsi = kpool.tile([128, 16], i32, tag="ofsi")
                    pixAP = _rp(WBv, 18, [[WSLOT, 16]])
                    nc.vector.tensor_scalar(ofsf[:], pixAP, 0.0, 4158.0, OP.max, OP.min)
                    nc.vector.tensor_copy(ofsi[:], ofsf[:])
                    # --- corner gather: 16 ops, 1KB/partition each (4 corners) ---
                    COR = gpool.tile([128, S, 1024], bf16, tag="COR")
                    for j in range(S):
                        cd = COR[:, j, :]
                        cd = dataclasses.replace(cd, ap=[cd.ap[0], [1, 1024]])
                        nc.gpsimd.indirect_dma_start(
                            cd, None, imgquad.ap(),
                            IndirectOffsetOnAxis(ap=ofsi[:, j:j + 1], axis=0))

                    hsr = kpool.tile([128, 256], f32, tag="hsr")
                    nc.scalar.dma_start(hsr[:], hsres.ap()[g * 128:(g + 1) * 128, :])

                    # --- bilinear: kv[j] = sum_c w_c * corner_c ---
                    kv = kpool.tile([128, S, 256], bf16, tag="kv")
                    for j in range(S):
                        nc.vector.tensor_scalar(kv[:, j, :], COR[:, j, 0:256],
                                                WB[:, j, 0:1], None, OP.mult)
                    for cc in (1, 2, 3):
                        for j in range(S):
                            nc.vector.scalar_tensor_tensor(
                                kv[:, j, :], COR[:, j, 256 * cc:256 * (cc + 1)], WB[:, j, cc:cc + 1],
                                kv[:, j, :], OP.mult, OP.add)

                    # --- scores ---
                    sc = kpool.tile([128, 64], f32, tag="sc")
                    scv = sc[:].rearrange("p (u a) -> p u a", a=4)
                    scr = spool.tile([128, 256], bf16, tag="scr")
                    for j in range(S):
                        for a in range(4):
                            nc.vector.scalar_tensor_tensor(
                                scr[:], QRb[:, j, 256 * a:256 * (a + 1)], 1.0, kv[:, j, :],
                                OP.mult, OP.mult,
                                accum_out=scv[:, j, a:a + 1])

                    # --- comb = gy*offy + gx*offx + ctil (baseline form) ---
                    comb = kpool.tile([128, S, 4], f32, tag="comb")
                    for j in range(S):
                        nc.vector.scalar_tensor_tensor(comb[:, j, :], WB[:, j, 6:10], WB[:, j, 4:5], WB[:, j, 14:18], OP.mult, OP.add)
                        nc.vector.scalar_tensor_tensor(comb[:, j, :], WB[:, j, 10:14], WB[:, j, 5:6], comb[:, j, :], OP.mult, OP.add)
                    nc.vector.tensor_tensor(sc[:], sc[:], comb[:].rearrange("p s a -> p (s a)"), OP.add)

                    # --- softmax over 16 samples (partition groups of 16) ---
                    pe_u = kpool.tile([128, 64], f32, tag="pe_u")
                    nc.scalar.activation(pe_u[:], sc[:], AF.Exp)
                    psZ = kpsum.tile([8, 64], f32, tag="psmA")
                    nc.tensor.matmul(psZ[:], c_m8[:], pe_u[:], start=True, stop=True)
                    rz = kpool.tile([8, 64], f32, tag="rz")
                    nc.vector.reciprocal(rz[:], psZ[:])
                    psR = kpsum.tile([128, 64], f32, tag="psmA")
                    nc.tensor.matmul(psR[:], c_bcm[:], rz[:], start=True, stop=True)
                    pn = kpool.tile([128, 64], bf16, tag="pn")
                    nc.vector.tensor_tensor(pn[:], pe_u[:], psR[:], OP.mult)

                    # --- offy/offx row for pyx matmuls ---
                    oyx = kpool.tile([128, S * 2], bf16, tag="oyx")
                    nc.vector.tensor_copy(oyx[:], _rp(WBv, 4, [[WSLOT, 16], [1, 2]]))

                    # --- attention-weighted sums on PE ---
                    ct0 = kpsum.tile([128, 512], f32, tag="ct0")
                    ct1 = kpsum.tile([128, 512], f32, tag="ct1")
                    pyxY = kpsum.tile([1, 512], f32, tag="pyxY")
                    pyxX = kpsum.tile([1, 512], f32, tag="pyxX")
                    for j in range(S):
                        pmj = kpool.tile([128, 32], bf16, tag="pm")
                        nc.vector.tensor_tensor(
                            pmj[:], _bcast(pn[:, j * 4:(j + 1) * 4], 8), c_m32[:], OP.mult)
                        pmu = pmj[:]
                        nc.tensor.matmul(ct0[:, j * 32:(j + 1) * 32],
                                         kv[:, j, 0:128], pmu, start=True, stop=True)
                        nc.tensor.matmul(ct1[:, j * 32:(j + 1) * 32],
                                         kv[:, j, 128:256], pmu, start=True, stop=True)
                        nc.tensor.matmul(pyxY[:, j * 32:(j + 1) * 32],
                                         oyx[:, 2 * j:2 * j + 1], pmu, start=True, stop=True)
                        nc.tensor.matmul(pyxX[:, j * 32:(j + 1) * 32],
                                         oyx[:, 2 * j + 1:2 * j + 2], pmu, start=True, stop=True)

                    ct0s = spool.tile([128, 512], bf16, tag="ct0s")
                    ct1s = spool.tile([128, 512], bf16, tag="ct1s")
                    pyxYs = kpool.tile([1, 512], bf16, tag="pyxYs")
                    pyxXs = kpool.tile([1, 512], bf16, tag="pyxXs")
                    nc.scalar.copy(ct0s[:], ct0[:])
                    nc.scalar.copy(ct1s[:], ct1[:])
                    nc.vector.tensor_copy(pyxYs[:], pyxY[:])
                    nc.vector.tensor_copy(pyxXs[:], pyxX[:])

                    # --- output projection: psAt[token, d] ---
                    psAt = kpsum.tile([128, 256], f32, tag="psmB")
                    first = True
                    for a in range(4):
                        for h in range(2):
                            cts = (ct0s, ct1s)[h]
                            lh = cts[:].rearrange("p (j t a) -> p (j t) a", j=S, a=4)
                            nc.tensor.matmul(psAt[:], lh[:, :, a],
                                             c_WvoK[a * 2 + h][:],
                                             start=first, stop=False)
                            first = False
                    for yx, pys in enumerate((pyxYs, pyxXs)):
                        lhp = pys[:].rearrange("p (j t a) -> p (j t) a", j=S, a=4)
                        for a in range(4):
                            nc.tensor.matmul(psAt[:], lhp[:, :, a],
                                             c_wyxvo[:, (yx * 4 + a) * 256:(yx * 4 + a + 1) * 256],
                                             start=False, stop=False)
                    nc.tensor.matmul(psAt[:], ones_bf[:], c_acst[:], start=False, stop=True)

                    xr = kpool.tile([128, 256], f32, tag="xr")
                    nc.vector.tensor_tensor(xr[:], hsr[:], psAt[:], OP.add)

                    def layernorm(xin, tagp):
                        sq = spool.tile([128, 256], f32, tag="lnsq")
                        ssq = kpool.tile([128, 1], f32, tag=tagp + "ssq")
                        nc.scalar.activation(sq[:], xin[:], AF.Square, accum_out=ssq[:])
                        sm = kpool.tile([128, 1], f32, tag=tagp + "sm")
                        nc.vector.tensor_reduce(sm[:], xin[:], axis=AX.X, op=OP.add)
                        mn = kpool.tile([128, 1], f32, tag=tagp + "mn")
                        nc.vector.tensor_scalar(mn[:], sm[:], 1.0 / 256.0, None, OP.mult)
                        msq = kpool.tile([128, 1], f32, tag=tagp + "msq")
                        nc.vector.tensor_tensor(msq[:], mn[:], mn[:], OP.mult)
                        vr = kpool.tile([128, 1], f32, tag=tagp + "vr")
                        nc.vector.scalar_tensor_tensor(vr[:], ssq[:], 1.0 / 256.0, msq[:], OP.mult, OP.subtract)
                        nc.vector.tensor_scalar(vr[:], vr[:], EPS, None, OP.add)
                        sd = kpool.tile([128, 1], f32, tag=tagp + "sd")
                        nc.scalar.activation(sd[:], vr[:], AF.Sqrt)
                        rstd = kpool.tile([128, 1], f32, tag=tagp + "rstd")
                        nc.vector.reciprocal(rstd[:], sd[:])
                        xo = kpool.tile([128, 256], f32, tag=tagp + "xo")
                        nc.vector.tensor_scalar(xo[:], xin[:], mn[:], rstd[:], OP.subtract, OP.mult)
                        return xo

                    xh = layernorm(xr, "ln1")
                    x1 = kpool.tile([128, 256], f32, tag="x1")
                    nc.vector.tensor_tensor(x1[:], xh[:], c_s1[:], OP.mult)
                    nc.vector.tensor_tensor(x1[:], x1[:], c_b1r[:], OP.add)

                    # transpose xh -> xT [256ch(2x128), 128t] bf16
                    psX = kpsum.tile([128, 256], f32, tag="psmA")
                    nc.tensor.transpose(psX[:, 0:128], xh[:, 0:128], c_id[:])
                    nc.tensor.transpose(psX[:, 128:256], xh[:, 128:256], c_id[:])
                    xT = kpool.tile([128, 256], bf16, tag="xT")
                    nc.scalar.copy(xT[:], psX[:])

                    # hT = W1'.T @ x1 in PSUM [hidden-sub, tok]
                    psH0 = kpsum.tile([128, 512], f32, tag="psH0")
                    psH1 = kpsum.tile([128, 512], f32, tag="psH1")
                    psH = [psH0, psH1]
                    for m in range(8):
                        pd = psH[m // 4][:, (m % 4) * 128:(m % 4 + 1) * 128]
                        for k in range(2):
                            nc.tensor.matmul(pd, c_W1[k][:, m * 128:(m + 1) * 128],
                                             xT[:, 128 * k:128 * (k + 1)],
                                             start=(k == 0), stop=False)
                        nc.tensor.matmul(pd, c_b1[:, m * 128:(m + 1) * 128], ones_bf[:],
                                         start=False, stop=True)
                    gh = spool.tile([128, MLP_H], bf16, tag="gh")
                    gsq = spool.tile([128, 512], f32, tag="gsq")
                    gu = spool.tile([128, 512], f32, tag="gu")
                    for mi in range(2):
                        hv = psH[mi][:]
                        nc.scalar.activation(gsq[:], hv, AF.Square)
                        nc.vector.tensor_scalar(gsq[:], gsq[:], 0.044715, 1.0, OP.mult, OP.add)
                        nc.vector.tensor_tensor(gu[:], gsq[:], hv, OP.mult)
                        nc.scalar.activation(gu[:], gu[:], AF.Tanh, scale=0.7978845608028654)
                        nc.vector.tensor_scalar(gu[:], gu[:], 0.5, 0.5, OP.mult, OP.add)
                        nc.vector.tensor_tensor(gh[:, mi * 512:(mi + 1) * 512], gu[:], hv, OP.mult)

                    psY = kpsum.tile([128, 256], f32, tag="psmB")
                    for k in range(8):
                        nc.tensor.matmul(psY[:], gh[:, k * 128:(k + 1) * 128],
                                         c_W2[k][:],
                                         start=(k == 0), stop=False)
                    nc.tensor.matmul(psY[:], ones_bf[:], c_b2[:], start=False, stop=True)

                    z = kpool.tile([128, 256], f32, tag="z")
                    nc.vector.tensor_tensor(z[:], x1[:], psY[:], OP.add)
                    xh2 = layernorm(z, "ln2")
                    yout = kpool.tile([128, 256], f32, tag="yout")
                    nc.vector.tensor_tensor(yout[:], xh2[:], c_s2[:], OP.mult)
                    nc.vector.tensor_tensor(yout[:], yout[:], c_b2r[:], OP.add)
                    nc.sync.dma_start(out.ap()[g * 128:(g + 1) * 128, :], yout[:])

    nc.compile()
    return nc


def _host_prep(inputs):
    f = np.float32
    hs = np.asarray(inputs['hidden_state'], f)
    ehs = np.asarray(inputs['embedded_hidden_state'], f)
    W_off = np.asarray(inputs['W_off'], f)      # [D, S, 2]
    b_off = np.asarray(inputs['b_off'], f)      # [S, 2]
    W_kvp = np.asarray(inputs['W_kvp'], f)      # [2, D]
    b_kvp = np.asarray(inputs['b_kvp'], f)      # [D]
    Wq = np.asarray(inputs['Wq'], f); bq = np.asarray(inputs['bq'], f)
    Wk = np.asarray(inputs['Wk'], f); bk = np.asarray(inputs['bk'], f)
    Wv = np.asarray(inputs['Wv'], f); bv = np.asarray(inputs['bv'], f)
    Wo = np.asarray(inputs['Wo'], f); bo = np.asarray(inputs['bo'], f)
    ln1_s = np.asarray(inputs['ln1_s'], f); ln1_b = np.asarray(inputs['ln1_b'], f)
    W1 = np.asarray(inputs['W1'], f); b1 = np.asarray(inputs['b1'], f)
    W2 = np.asarray(inputs['W2'], f); b2 = np.asarray(inputs['b2'], f)
    ln2_s = np.asarray(inputs['ln2_s'], f); ln2_b = np.asarray(inputs['ln2_b'], f)

    sc = 1.0 / np.sqrt(HD)
    Wtil = np.zeros((256, 4, 256), f)
    btilv = np.zeros((4, 256), f)
    gyv = np.zeros((256, 4), f); gxv = np.zeros((256, 4), f); cv = np.zeros((256, 4), f)
    gyb = np.zeros(4, f); gxb = np.zeros(4, f); cb = np.zeros(4, f)
    WvoK = np.zeros((4, 256, 256), f)
    wyxvo = np.zeros((8, 256), f)
    acst = np.array(bo, f)
    for a in range(4):
        Wt = (Wq[:, a, :] @ Wk[:, a, :].T) * sc
        bt = (bq[a] @ Wk[:, a, :].T) * sc
        Wtil[:, a, :] = Wt
        btilv[a] = bt
        gyv[:, a] = Wt @ W_kvp[0]; gyb[a] = bt @ W_kvp[0]
        gxv[:, a] = Wt @ W_kvp[1]; gxb[a] = bt @ W_kvp[1]
        kb = b_kvp @ Wk[:, a, :] + bk[a]
        cv[:, a] = (Wq[:, a, :] @ kb) * sc
        cb[a] = (bq[a] @ kb) * sc
        Wvo = Wv[:, a, :] @ Wo[a]
        WvoK[a] = Wvo
        wyxvo[a] = W_kvp[0] @ Wvo
        wyxvo[4 + a] = W_kvp[1] @ Wvo
        acst = acst + (b_kvp @ Wv[:, a, :] + bv[a]) @ Wo[a]

    Woff_flat = np.concatenate(
        [W_off[:, :, 0], W_off[:, :, 1], gyv, gxv, cv], axis=1)    # [256, 44]
    baug = np.concatenate(
        [b_off[:, 0], b_off[:, 1], gyb, gxb, cb])[None, :]         # [1, 44]
    W1p = ln1_s[:, None] * W1
    b1p = (ln1_b @ W1 + b1)[None, :]

    shared = {
        'phA_rhs': Woff_flat.astype(BF), 'phA_aug': baug.astype(BF),
        'Wtil': Wtil.reshape(256, 1024).astype(BF),
        'btil': btilv.reshape(1, 1024).astype(BF),
        'WvoK': WvoK.reshape(1024, 256).astype(BF),
        'wyxvo': wyxvo.reshape(1, 2048).astype(BF), 'acst': acst[None, :].astype(BF),
        'W1p': W1p.astype(BF), 'b1p': b1p.astype(BF),
        'W2w': W2.astype(BF), 'b2a': b2[None, :].astype(BF),
        's1rep': np.tile(ln1_s, (128, 1)).astype(f),
        'b1rep': np.tile(ln1_b, (128, 1)).astype(f),
        's2rep': np.tile(ln2_s, (128, 1)).astype(f),
        'b2rep': np.tile(ln2_b, (128, 1)).astype(f),
        'ident': np.eye(128, dtype=f),
        'mask8f': np.repeat(np.eye(8, dtype=f), 16, axis=0),
        'bcmask': np.repeat(np.eye(8, dtype=f), 16, axis=0).T.copy(),
        'mask32': np.repeat(np.repeat(np.eye(8, dtype=f), 16, axis=0), 4, axis=1).astype(BF),
        'ioQ2': (16 * (np.arange(128)[:, None] // 16)).astype(np.int32),
    }
    tok = np.arange(T)
    refy_all = (tok // 64).astype(f)
    refx_all = (tok % 64).astype(f) - 30.0

    in_maps = []
    for c in range(NCORES):
        n, r0 = c // 4, (c % 4) * 16
        hs_c = hs[n, r0:r0 + 16].reshape(T, 256)
        img = ehs[n]
        P = np.zeros((65, 65, 256), f)
        P[:64, :64] = img
        P[64, :64] = img[63]
        P[:64, 64] = P[:64, 63]
        P[64, 64] = img[63, 63]
        m = dict(shared)
        m['hsT'] = np.ascontiguousarray(hs_c.T).astype(BF)
        # permuted copy for the Q path: chunk-local col p <- token 8*(p%16)+p//16
        pa = np.arange(128)
        perm = (np.arange(T).reshape(NT, 128)[:, 8 * (pa % 16) + pa // 16]).reshape(T)
        m['hsT2'] = np.ascontiguousarray(hs_c[perm].T).astype(BF)
        m['hsres'] = hs_c.copy()
        Pf = P.reshape(NPIX, 256)
        iq = np.zeros((4160, 4, 256), f)
        kk = np.arange(4159)
        iq[kk, 0] = Pf[kk]
        iq[kk, 1] = Pf[kk + 1]
        iq[kk, 2] = Pf[kk + 65]
        iq[kk, 3] = Pf[kk + 66]
        m['imgquad'] = iq.reshape(4160, 1024).astype(BF)
        m['refy'] = np.ascontiguousarray(
            (refy_all + r0 - 30.0).reshape(8, 128).T)
        m['refx'] = np.ascontiguousarray(refx_all.reshape(8, 128).T)
        in_maps.append(m)
    return in_maps


def kernel(**inputs):
    from concourse.bass_utils import run_bass_kernel_spmd
    if 'nc' not in _CACHE:
        _CACHE['nc'] = _build()
    nc = _CACHE['nc']
    in_maps = _host_prep(inputs)
    res = run_bass_kernel_spmd(nc, in_maps, list(range(NCORES)))
    outs = [res.results[c]['out'].reshape(16, 64, 256) for c in range(NCORES)]
    full = np.zeros((N, H, W, D), np.float32)
    for c in range(NCORES):
        full[c // 4, (c % 4) * 16:(c % 4) * 16 + 16] = outs[c]
    return full
